# revision 2
# baseline (speedup 1.0000x reference)
"""BotRGCN on 8 Trainium2 NeuronCores (Bass/Tile) — wire-optimized.

The wall-clock is dominated by the axon tunnel (~25-50 MB/s raw, no
compression, ~0.1s fixed dispatch+fetch): the kernel minimizes raw wire
bytes and ships them in two large arrays.

  - Host: encoder MLP in f32; x quantized AFFINE per-feature to 127
    levels and bit-packed 7-bit (8 codes -> 7 byte-planes of 16 cols,
    node-major).  The induced feature permutation is folded into the
    weights on host.  Device unpacks with shift/or ops on contiguous
    [128,16] slices; the gather table holds raw integer codes (exact in
    bf16), dequant (s, zp) happens after aggregation.
  - Graph: nodes dealt to 8*BPC blocks of 128 dst lanes (serpentine on
    degree).  Edges bucketed per (dst block, rel); rel is implicit in
    the tile index (per-(block,rel) tile counts, variable; blocks
    sorted per core by bucket size so the SPMD max across cores stays
    tight).  Per edge slot 3 bytes in separate planes: row_lo, row_mid,
    lane|row_hi<<7.  Padding slots point to a reserved all-zero table
    row (code 0 contributes nothing).
  - Mean normalization: per-(lane,rel) counts ship as u8; device takes
    f32 reciprocal (exact) and applies it via a rank-1 outer-product
    matmul + tensor_tensor after PSUM aggregation; affine dequant is
    one fused tensor_scalar.
  - Edge planes + counts + weights (bf16, replicated) + f32 biases/
    scales ride in ONE merged u8 "meta" array per core (bitcast slices
    on device) — few large transfers beat many small ones.
  - Numerics: aggregation exact (int codes / 0-1 selectors in bf16,
    f32 PSUM); transforms + head in f32 (weights rounded to bf16 on the
    wire only); inter-layer table in bf16.  Measured end-to-end error:
    l2 6.5e-3, max 1.6e-2 (gate 2e-2).
"""

import numpy as np
import ml_dtypes

import jax
from jax.sharding import Mesh, PartitionSpec
from jax.experimental.shard_map import shard_map

import concourse.bacc as bacc
import concourse.bass as bass
import concourse.bass2jax as b2j
import concourse.mybir as mybir
import concourse.tile as tile
from concourse.masks import make_identity

F32 = mybir.dt.float32
BF16 = mybir.dt.bfloat16
I32 = mybir.dt.int32
U8 = mybir.dt.uint8
BF = ml_dtypes.bfloat16

N_CORES = 8
D = 128
R = 2
ALPHA = 0.01
WCOLS = 524  # Wroot|Wrel0|Wrel1|Wo1 (4*128) | Wo2 (2) | f32 brg|bo1|bo2|s|zp (10 bf16)


def _lrelu_np(v):
    return np.where(v > 0, v, np.float32(ALPHA) * v)


# ----------------------------------------------------------------------------
# host-side preprocessing
# ----------------------------------------------------------------------------

def _prep(inputs):
    src = np.asarray(inputs["edge_index"][0], dtype=np.int64)
    dst = np.asarray(inputs["edge_index"][1], dtype=np.int64)
    rel = np.asarray(inputs["edge_type"], dtype=np.int64)
    N = int(np.asarray(inputs["des"]).shape[0])
    E = src.shape[0]

    BPC = ((-(-N // N_CORES)) + 1 + 127) // 128  # >=1 spare slot per core
    SHARD = BPC * 128
    NBLK = N_CORES * BPC
    TROWS = N_CORES * SHARD
    assert TROWS <= (1 << 17)

    cnt2 = np.bincount(dst * R + rel, minlength=N * R).reshape(N, R)
    deg = cnt2.sum(1)

    # serpentine deal by degree
    order = np.argsort(-deg, kind="stable")
    idx = np.arange(N)
    rnd = idx // NBLK
    pos = idx % NBLK
    blk = np.where(rnd % 2 == 0, pos, NBLK - 1 - pos)
    node_block = np.empty(N, np.int64)
    node_lane = np.empty(N, np.int64)
    node_block[order] = blk
    node_lane[order] = rnd
    assert node_lane.max() <= 127

    # per-core block relabel: sort blocks by bucket size so per-position
    # maxima across cores stay tight (variable tile counts)
    key0 = node_block[dst]
    cnt_b0 = np.bincount(key0, minlength=NBLK)
    newid = np.empty(NBLK, np.int64)
    for c in range(N_CORES):
        sl = slice(c * BPC, (c + 1) * BPC)
        rank = np.argsort(-cnt_b0[sl], kind="stable")
        newid[c * BPC + rank] = c * BPC + np.arange(BPC)
    node_block = newid[node_block]

    # relocate nodes off the reserved (new last block of core, lane 127)
    # slots — these rows must stay all-zero in both gather tables
    used = np.zeros((NBLK, 128), bool)
    used[node_block, node_lane] = True
    res_blocks = np.arange(N_CORES) * BPC + BPC - 1
    hit = np.where(np.isin(node_block, res_blocks) & (node_lane == 127))[0]
    free_mask = ~used
    free_mask[res_blocks, 127] = False
    free_list = np.argwhere(free_mask)
    assert len(free_list) >= len(hit)
    for j, i in enumerate(hit):
        node_block[i], node_lane[i] = free_list[j]

    node_core = node_block // BPC
    node_pos = (node_block % BPC) * 128 + node_lane
    node_row = node_core * SHARD + node_pos
    res_row = np.arange(N_CORES) * SHARD + SHARD - 1  # [8]

    key = node_block[dst]
    cnt_br = np.bincount(key * R + rel, minlength=NBLK * R)  # (block, rel)
    cnt_pos = cnt_br.reshape(N_CORES, BPC, R)
    Tbr = np.maximum(-(-cnt_pos.max(axis=0) // 128), 1)  # [BPC, R]
    tile_off = np.zeros(BPC * R + 1, np.int64)
    tile_off[1:] = np.cumsum(Tbr.reshape(-1))
    NT2 = int(tile_off[-1])

    lane_e = node_lane[dst]
    rowg = node_row[src]
    order_e = np.lexsort((rowg, lane_e, rel, key))
    ks, rs = key[order_e], rel[order_e]
    seg = ks * R + rs
    starts = np.zeros(NBLK * R, np.int64)
    starts[1:] = np.cumsum(cnt_br)[:-1]
    pos_in = np.arange(E) - starts[seg]

    core_e = ks // BPC
    posb_e = ks % BPC
    col_e = tile_off[posb_e * R + rs] + pos_in // 128
    par_e = pos_in % 128

    rows_all = np.broadcast_to(res_row[:, None, None], (N_CORES, NT2, 128)).copy()
    lanes_all = np.zeros((N_CORES, NT2, 128), np.int64)
    rows_all[core_e, col_e, par_e] = rowg[order_e]
    lanes_all[core_e, col_e, par_e] = lane_e[order_e]

    def planes(a):
        return np.ascontiguousarray(a.transpose(0, 2, 1)).astype(np.uint8)

    ep0 = planes(rows_all & 255)
    ep1 = planes((rows_all >> 8) & 255)
    ep2 = planes(lanes_all | ((rows_all >> 16) << 7))

    cnt_nl = np.zeros((NBLK, 128, R), np.int64)
    cnt_nl[node_block, node_lane] = cnt2
    assert cnt_nl.max() <= 255
    # [core, lane, r*BPC + b] u8
    cntw = np.ascontiguousarray(
        np.maximum(cnt_nl, 1).reshape(N_CORES, BPC, 128, R)
        .transpose(0, 2, 3, 1).reshape(N_CORES, 128, R * BPC)).astype(np.uint8)

    # host encoder + affine 127-level quantization
    g = lambda k: np.asarray(inputs[k], np.float32)
    d = _lrelu_np(g("des") @ g("W_des") + g("b_des"))
    t = _lrelu_np(g("tweet") @ g("W_tweet") + g("b_tweet"))
    n = _lrelu_np(g("num_prop") @ g("W_num") + g("b_num"))
    c = _lrelu_np(g("cat_prop") @ g("W_cat") + g("b_cat"))
    x = _lrelu_np(np.concatenate([d, t, n, c], axis=1) @ g("W_in") + g("b_in"))
    lo, hi = x.min(0), x.max(0)
    xs = np.maximum(hi - lo, 1e-12).astype(np.float32) / 127.0
    zp = lo.astype(np.float32)
    codes = np.clip(np.rint((x - zp) / xs), 0, 127).astype(np.uint8)

    # feature permutation induced by the 7-bit plane decode:
    # new feature 16*k + g  <-  original feature 8*g + k
    kk, gg = np.meshgrid(np.arange(8), np.arange(16), indexing="ij")
    perm = (8 * gg + kk).reshape(-1)  # [128] new -> old

    # 7-bit pack: group g = original features 8g..8g+7 -> 7 bytes, laid
    # as 7 planes of 16 cols (wire col = 16*p + g)
    acc = np.zeros((N, 16), np.uint64)
    for k in range(8):
        acc |= codes[:, k::8].astype(np.uint64) << np.uint64(7 * k)
    x7 = np.empty((N, 112), np.uint8)
    for p in range(7):
        x7[:, 16 * p:16 * (p + 1)] = (acc >> np.uint64(8 * p)).astype(np.uint8)

    row_node = np.full(TROWS, -1, np.int64)
    row_node[node_row] = np.arange(N)
    valid = row_node >= 0
    Xr = x7[np.where(valid, row_node, 0)]
    Xr[~valid] = 0
    xqf = np.ascontiguousarray(Xr.reshape(N_CORES, SHARD, 112))

    # weights: bf16, replicated; scales/biases f32 bitcast into bf16 cols.
    # All feature-indexed tensors permuted to the decode order.
    Wroot = g("W_root")[perm][:, perm]
    Wrel0 = g("W_rel")[0][perm][:, perm]
    Wrel1 = g("W_rel")[1][perm][:, perm]
    Wo1 = g("W_o1")[perm]
    Wo2 = g("W_o2")
    wcore = np.zeros((128, WCOLS), BF)
    wcore[:, 0:128] = Wroot.astype(BF)
    wcore[:, 128:256] = Wrel0.astype(BF)
    wcore[:, 256:384] = Wrel1.astype(BF)
    wcore[:, 384:512] = Wo1.astype(BF)
    wcore[:, 512:514] = Wo2.astype(BF)
    f32sec = np.zeros((128, 5), np.float32)
    f32sec[:, 0] = g("b_rgcn")[perm]
    f32sec[:, 1] = g("b_o1")
    f32sec[0:2, 2] = g("b_o2")
    f32sec[:, 3] = xs[perm]
    f32sec[:, 4] = zp[perm]
    wcore[:, 514:524] = f32sec.view(BF)
    wts = np.broadcast_to(wcore, (N_CORES, 128, WCOLS)).copy()

    # merge edge planes + counts + weights into one wire array per core
    # (few large transfers beat many small ones on the tunnel)
    WOFF = ((3 * NT2 + 2 * BPC + 3) // 4) * 4
    M = WOFF + 2 * WCOLS
    meta = np.zeros((N_CORES, 128, M), np.uint8)
    meta[:, :, 0:NT2] = ep0
    meta[:, :, NT2:2 * NT2] = ep1
    meta[:, :, 2 * NT2:3 * NT2] = ep2
    meta[:, :, 3 * NT2:3 * NT2 + 2 * BPC] = cntw
    meta[:, :, WOFF:] = wts.view(np.uint8)

    cfg = dict(N=N, E=E, BPC=BPC, SHARD=SHARD, TROWS=TROWS, NT2=NT2,
               tiles=tuple(map(tuple, Tbr.tolist())))
    per_core = dict(xq=xqf, meta=meta)
    asm = dict(node_core=node_core, node_pos=node_pos)
    return cfg, per_core, asm


# ----------------------------------------------------------------------------
# device program
# ----------------------------------------------------------------------------

def _enc_slices(shard, w_max=512):
    out, c = [], 0
    while c < shard:
        w = min(w_max, shard - c)
        out.append((c, w))
        c += w
    return out


def build_bass(cfg, sim_compat=False):
    BPC, SHARD, TROWS, NT2 = cfg["BPC"], cfg["SHARD"], cfg["TROWS"], cfg["NT2"]
    tiles = cfg["tiles"]
    tile_off = [0]
    for b in range(BPC):
        for r in range(R):
            tile_off.append(tile_off[-1] + tiles[b][r])

    nc = bacc.Bacc("TRN2", target_bir_lowering=False, debug=False,
                   num_devices=N_CORES)

    WOFF = ((3 * NT2 + 2 * BPC + 3) // 4) * 4
    M = WOFF + 2 * WCOLS
    xq = nc.dram_tensor("xq", [SHARD, 112], U8, kind="ExternalInput")
    meta = nc.dram_tensor("meta", [128, M], U8, kind="ExternalInput")
    out = nc.dram_tensor("out", [2, SHARD], BF16, kind="ExternalOutput")
    ep0 = meta[:, 0:NT2]
    ep1 = meta[:, NT2:2 * NT2]
    ep2 = meta[:, 2 * NT2:3 * NT2]
    cnt = meta[:, 3 * NT2:3 * NT2 + 2 * BPC]
    wtsb = meta[:, WOFF:WOFF + 2 * 514].bitcast(BF16)
    wtsf = meta[:, WOFF + 2 * 514:WOFF + 2 * 514 + 20].bitcast(F32)

    groups = [list(range(N_CORES))]
    AG = "AllGather"
    BY = mybir.AluOpType.bypass

    def _lrelu(pool, ps_ap, bias_ap, w, name):
        t = pool.tile([ps_ap.shape[0], w], F32, name=name)
        if not sim_compat:
            nc.scalar.activation(out=t[:], in_=ps_ap,
                                 func=mybir.ActivationFunctionType.Prelu,
                                 bias=bias_ap, scale=1.0, alpha=ALPHA)
            return t
        zt = pool.tile([ps_ap.shape[0], w], F32, name=name + "_z")
        nc.scalar.activation(out=zt[:], in_=ps_ap,
                             func=mybir.ActivationFunctionType.Identity,
                             bias=bias_ap, scale=1.0)
        rt = pool.tile([ps_ap.shape[0], w], F32, name=name + "_r")
        nc.scalar.activation(out=rt[:], in_=ps_ap,
                             func=mybir.ActivationFunctionType.Relu,
                             bias=bias_ap, scale=1.0)
        t1 = pool.tile([ps_ap.shape[0], w], F32, name=name + "_t1")
        nc.vector.tensor_scalar(out=t1[:], in0=zt[:], scalar1=ALPHA, scalar2=None,
                                op0=mybir.AluOpType.mult)
        t2 = pool.tile([ps_ap.shape[0], w], F32, name=name + "_t2")
        nc.vector.tensor_scalar(out=t2[:], in0=rt[:], scalar1=1.0 - ALPHA,
                                scalar2=None, op0=mybir.AluOpType.mult)
        nc.vector.tensor_tensor(out=t[:], in0=t1[:], in1=t2[:],
                                op=mybir.AluOpType.add)
        return t

    with tile.TileContext(nc) as tc:
        with tc.tile_pool(name="const", bufs=1) as cp, \
             tc.tile_pool(name="dram", bufs=1, space="DRAM") as dp:
            # ---- weights: replicated bf16 wire, f32 on device ----
            c_Wb = cp.tile([128, 514], BF16)
            nc.sync.dma_start(c_Wb[:], wtsb)
            c_W = cp.tile([128, 514], F32)
            nc.vector.tensor_copy(out=c_W[:], in_=c_Wb[:])
            c_f32 = cp.tile([128, 5], F32)
            nc.sync.dma_start(c_f32[:], wtsf)
            c_Wroot = c_W[:, 0:128]
            c_Wrel = [c_W[:, 128:256], c_W[:, 256:384]]
            c_Wo1 = c_W[:, 384:512]
            c_Wo2 = c_W[:, 512:514]
            c_brg = c_f32[:, 0:1]
            c_bo1 = c_f32[:, 1:2]
            c_bo2 = c_f32[0:2, 2:3]
            c_s = c_f32[:, 3:4]
            c_zp = c_f32[:, 4:5]

            ident = cp.tile([128, 128], BF16)
            make_identity(nc, ident[:])

            # ---- edge metadata decode ----
            with tc.tile_pool(name="dec", bufs=1) as dcp, \
                 tc.tile_pool(name="decps", bufs=2, space="PSUM") as dps:
                p0 = dcp.tile([128, NT2], U8)
                p1 = dcp.tile([128, NT2], U8)
                p2 = dcp.tile([128, NT2], U8)
                nc.sync.dma_start(p0[:], ep0)
                nc.sync.dma_start(p1[:], ep1)
                nc.sync.dma_start(p2[:], ep2)
                g0 = dcp.tile([128, NT2], I32)
                nc.vector.tensor_copy(out=g0[:], in_=p0[:])
                g1 = dcp.tile([128, NT2], I32)
                nc.vector.tensor_copy(out=g1[:], in_=p1[:])
                g2 = dcp.tile([128, NT2], I32)
                nc.vector.tensor_copy(out=g2[:], in_=p2[:])
                t1_ = dcp.tile([128, NT2], I32)
                nc.vector.tensor_scalar(out=t1_[:], in0=g1[:], scalar1=8,
                                        scalar2=None,
                                        op0=mybir.AluOpType.logical_shift_left)
                t2_ = dcp.tile([128, NT2], I32)
                nc.vector.tensor_scalar(out=t2_[:], in0=g2[:], scalar1=7,
                                        scalar2=16,
                                        op0=mybir.AluOpType.logical_shift_right,
                                        op1=mybir.AluOpType.logical_shift_left)
                c_gidx = cp.tile([128, NT2], I32)
                nc.vector.tensor_tensor(out=c_gidx[:], in0=g0[:], in1=t1_[:],
                                        op=mybir.AluOpType.add)
                nc.vector.tensor_tensor(out=c_gidx[:], in0=c_gidx[:], in1=t2_[:],
                                        op=mybir.AluOpType.add)
                lf = dcp.tile([128, NT2], I32)
                nc.vector.tensor_scalar(out=lf[:], in0=g2[:], scalar1=127,
                                        scalar2=None,
                                        op0=mybir.AluOpType.bitwise_and)
                c_lane = cp.tile([128, NT2], F32)
                nc.vector.tensor_copy(out=c_lane[:], in_=lf[:])

                cb = dcp.tile([128, R * BPC], U8)
                nc.sync.dma_start(cb[:], cnt)
                cf = dcp.tile([128, R * BPC], BF16)
                nc.vector.tensor_copy(out=cf[:], in_=cb[:])
                c_wvd = dp.tile([R * BPC, 128], F32, name="wvd")
                for r in range(R):
                    pst = dps.tile([BPC, 128], BF16, name="cwps")
                    nc.tensor.matmul(out=pst[:], lhsT=cf[:, r * BPC:(r + 1) * BPC],
                                     rhs=ident[:], is_transpose=True,
                                     start=True, stop=True)
                    cg = dcp.tile([BPC, 128], F32, name=f"cntT{r}")
                    nc.vector.tensor_copy(out=cg[:], in_=pst[:])
                    wvr = dcp.tile([BPC, 128], F32, name=f"wvT{r}")
                    nc.vector.reciprocal(out=wvr[:], in_=cg[:])
                    nc.sync.dma_start(c_wvd[r * BPC:(r + 1) * BPC, :], wvr[:])

            c_ioti = cp.tile([128, 128], I32)
            nc.gpsimd.iota(c_ioti[:], pattern=[[1, 128]], base=0,
                           channel_multiplier=0)
            c_iota = cp.tile([128, 128], F32)
            nc.vector.tensor_copy(out=c_iota[:], in_=c_ioti[:])
            c_ones = cp.tile([1, 128], F32)
            nc.vector.memset(c_ones[:], 1.0)
            zrow = cp.tile([1, 128], BF16)
            nc.vector.memset(zrow[:], 0.0)

            # DRAM intermediates
            xfm = [dp.tile([128, SHARD], F32, name=f"xfm{i}") for i in range(3)]
            xnm = [dp.tile([SHARD, 128], BF16, name=f"xnm{i}") for i in range(2)]
            tables = [dp.tile([TROWS, 128], BF16, addr_space="Shared",
                              name=f"table{i}") for i in range(2)]

            # ---- ingest: unpack 7-bit codes -> table codes + dequant x ----
            OR_ = mybir.AluOpType.bitwise_or
            AND = mybir.AluOpType.bitwise_and
            LSR = mybir.AluOpType.logical_shift_right
            LSL = mybir.AluOpType.logical_shift_left
            with tc.tile_pool(name="ing", bufs=4) as ip, \
                 tc.tile_pool(name="ingps", bufs=2, space="PSUM") as ips:
                for k in range(BPC):
                    raw = ip.tile([128, 112], U8, name="raw")
                    nc.sync.dma_start(raw[:], xq[k * 128:(k + 1) * 128, :])
                    dec = ip.tile([128, 128], U8, name="dec")
                    for j in range(8):
                        a, sa = (7 * j) >> 3, (7 * j) & 7
                        dst = dec[:, 16 * j:16 * (j + 1)]
                        if sa == 0:
                            nc.vector.tensor_scalar(
                                out=dst, in0=raw[:, 16 * a:16 * (a + 1)],
                                scalar1=127, scalar2=None, op0=AND)
                        elif a == 6:
                            nc.vector.tensor_scalar(
                                out=dst, in0=raw[:, 16 * a:16 * (a + 1)],
                                scalar1=sa, scalar2=None, op0=LSR)
                        else:
                            tlo = ip.tile([128, 16], U8, name="tlo")
                            nc.vector.tensor_scalar(
                                out=tlo[:], in0=raw[:, 16 * a:16 * (a + 1)],
                                scalar1=sa, scalar2=None, op0=LSR)
                            thi = ip.tile([128, 16], U8, name="thi")
                            nc.vector.tensor_scalar(
                                out=thi[:], in0=raw[:, 16 * (a + 1):16 * (a + 2)],
                                scalar1=8 - sa, scalar2=127, op0=LSL, op1=AND)
                            nc.vector.tensor_tensor(out=dst, in0=tlo[:],
                                                    in1=thi[:], op=OR_)
                    nb = ip.tile([128, 128], BF16, name="nb")
                    nc.vector.tensor_copy(out=nb[:], in_=dec[:])
                    nc.sync.dma_start(xnm[0][k * 128:(k + 1) * 128, :], nb[:])
                    ps_t = ips.tile([128, 128], BF16, name="ps_t")
                    nc.tensor.matmul(out=ps_t[:], lhsT=nb[:], rhs=ident[:],
                                     is_transpose=True, start=True, stop=True)
                    fm = ip.tile([128, 128], F32, name="fm")
                    nc.vector.tensor_scalar(out=fm[:], in0=ps_t[:],
                                            scalar1=c_s, scalar2=c_zp,
                                            op0=mybir.AluOpType.mult,
                                            op1=mybir.AluOpType.add)
                    nc.sync.dma_start(xfm[0][:, k * 128:(k + 1) * 128], fm[:])

            nc.gpsimd.collective_compute(AG, BY, replica_groups=groups,
                                         ins=[xnm[0].opt()], outs=[tables[0].opt()])

            # ---- rgcn layers ----
            for L in range(2):
                table = tables[L]
                Wroot_L, Wrel_L, bias_L = c_Wroot, c_Wrel, c_brg
                with tc.tile_pool(name=f"gp{L}", bufs=16) as gp, \
                     tc.tile_pool(name=f"sp{L}", bufs=8) as sp, \
                     tc.tile_pool(name=f"up{L}", bufs=4) as up, \
                     tc.tile_pool(name=f"Sps{L}", bufs=2, space="PSUM") as Sps, \
                     tc.tile_pool(name=f"Wps{L}", bufs=2, space="PSUM") as Wps, \
                     tc.tile_pool(name=f"aps{L}", bufs=2, space="PSUM") as aps, \
                     tc.tile_pool(name=f"tps{L}", bufs=2, space="PSUM") as tps:
                    n_units = BPC // 2 + (BPC % 2)
                    for u in range(n_units):
                        blocks = [b for b in (2 * u, 2 * u + 1) if b < BPC]
                        Us = []
                        for b in blocks:
                            ps = Sps.tile([128, 256], F32, name="psS")
                            for r in range(R):
                                Tb = tiles[b][r]
                                base = tile_off[b * R + r]
                                for t in range(Tb):
                                    T = base + t
                                    G = gp.tile([128, 128], BF16, name="G")
                                    nc.gpsimd.indirect_dma_start(
                                        out=G[:], out_offset=None, in_=table[:],
                                        in_offset=bass.IndirectOffsetOnAxis(
                                            ap=c_gidx[:, T:T + 1], axis=0))
                                    sel = sp.tile([128, 128], BF16, name="sel")
                                    nc.vector.tensor_scalar(
                                        out=sel[:], in0=c_iota[:],
                                        scalar1=c_lane[:, T:T + 1], scalar2=None,
                                        op0=mybir.AluOpType.is_equal)
                                    nc.tensor.matmul(
                                        out=ps[:, r * 128:(r + 1) * 128],
                                        lhsT=G[:], rhs=sel[:],
                                        start=(t == 0), stop=(t == Tb - 1))
                            U = up.tile([128, 256], F32, name="U")
                            for r in range(R):
                                wrow = sp.tile([1, 128], F32, name="wrow")
                                nc.sync.dma_start(wrow[:],
                                                  c_wvd[r * BPC + b:r * BPC + b + 1, :])
                                wvt = Wps.tile([128, 128], F32, name="wvt")
                                nc.tensor.matmul(
                                    out=wvt[:], lhsT=c_ones[:], rhs=wrow[:],
                                    start=True, stop=True)
                                wvs = sp.tile([128, 128], F32, name="wvs")
                                nc.vector.tensor_copy(out=wvs[:], in_=wvt[:])
                                nc.vector.tensor_tensor(
                                    out=U[:, r * 128:(r + 1) * 128],
                                    in0=ps[:, r * 128:(r + 1) * 128],
                                    in1=wvs[:], op=mybir.AluOpType.mult)
                            if L == 0:
                                # dequant the aggregated code-means
                                Ud = up.tile([128, 256], F32, name="Ud")
                                nc.vector.tensor_scalar(
                                    out=Ud[:], in0=U[:], scalar1=c_s,
                                    scalar2=c_zp, op0=mybir.AluOpType.mult,
                                    op1=mybir.AluOpType.add)
                                U = Ud
                            Us.append(U)
                        w = 128 * len(blocks)
                        c0 = u * 256
                        xr = up.tile([128, w], F32, name="xr")
                        nc.sync.dma_start(xr[:], xfm[L][:, c0:c0 + w])
                        agg = aps.tile([128, w], F32, name="agg")
                        nc.tensor.matmul(out=agg[:], lhsT=Wroot_L, rhs=xr[:],
                                         start=True, stop=False)
                        for h, b in enumerate(blocks):
                            last = (h == len(blocks) - 1)
                            for r in range(R):
                                nc.tensor.matmul(
                                    out=agg[:, h * 128:(h + 1) * 128],
                                    lhsT=Wrel_L[r],
                                    rhs=Us[h][:, r * 128:(r + 1) * 128],
                                    start=False,
                                    stop=(last and r == R - 1))
                        y = up.tile([128, w], F32, name="y")
                        nc.scalar.activation(out=y[:], in_=agg[:],
                                             func=mybir.ActivationFunctionType.Identity,
                                             bias=bias_L, scale=1.0)
                        nc.sync.dma_start(xfm[L + 1][:, c0:c0 + w], y[:])
                        if L == 0:
                            yb = up.tile([128, w], BF16, name="yb")
                            nc.vector.tensor_copy(out=yb[:], in_=y[:])
                            for j in range(len(blocks)):
                                ps_t = tps.tile([128, 128], BF16, name="ps_t2")
                                nc.tensor.matmul(
                                    out=ps_t[:],
                                    lhsT=yb[:, j * 128:(j + 1) * 128],
                                    rhs=ident[:], is_transpose=True,
                                    start=True, stop=True)
                                tr_t = up.tile([128, 128], BF16, name="tr2")
                                nc.vector.tensor_copy(out=tr_t[:], in_=ps_t[:])
                                nc.sync.dma_start(
                                    xnm[1][c0 + j * 128:c0 + (j + 1) * 128, :],
                                    tr_t[:])
                if L == 0:
                    nc.sync.dma_start(xnm[1][SHARD - 1:SHARD, :], zrow[:])
                    nc.gpsimd.collective_compute(AG, BY, replica_groups=groups,
                                                 ins=[xnm[1].opt()],
                                                 outs=[tables[1].opt()])

            # ---- head ----
            with tc.tile_pool(name="hd", bufs=3) as hp, \
                 tc.tile_pool(name="hps", bufs=2, space="PSUM") as hps, \
                 tc.tile_pool(name="ops", bufs=2, space="PSUM") as ops:
                for (c0, w) in _enc_slices(SHARD):
                    xt = hp.tile([128, w], F32, name="xt")
                    nc.sync.dma_start(xt[:], xfm[2][:, c0:c0 + w])
                    ps_h = hps.tile([128, w], F32, name="ps_h")
                    nc.tensor.matmul(out=ps_h[:], lhsT=c_Wo1, rhs=xt[:],
                                     start=True, stop=True)
                    z_t = _lrelu(hp, ps_h[:], c_bo1, w, "z_t")
                    ps_o = ops.tile([2, w], F32, name="ps_o")
                    nc.tensor.matmul(out=ps_o[:], lhsT=c_Wo2, rhs=z_t[:],
                                     start=True, stop=True)
                    o_t = hp.tile([2, w], BF16, name="o_t")
                    nc.scalar.activation(out=o_t[:], in_=ps_o[:],
                                         func=mybir.ActivationFunctionType.Identity,
                                         bias=c_bo2, scale=1.0)
                    nc.sync.dma_start(out[:, c0:c0 + w], o_t[:])
    nc.compile()
    return nc


# ----------------------------------------------------------------------------
# cached PJRT runner (unchanged from v1)
# ----------------------------------------------------------------------------

class _Runner:
    def __init__(self, cfg):
        self.cfg = cfg
        self.nc = build_bass(cfg)
        b2j.install_neuronx_cc_hook()
        nc = self.nc
        partition_name = (nc.partition_id_tensor.name
                          if nc.partition_id_tensor else None)
        in_names, out_names, out_avals = [], [], []
        for alloc in nc.m.functions[0].allocations:
            if not isinstance(alloc, mybir.MemoryLocationSet):
                continue
            name = alloc.memorylocations[0].name
            if alloc.kind == "ExternalInput":
                if name != partition_name:
                    in_names.append(name)
            elif alloc.kind == "ExternalOutput":
                shape = tuple(alloc.tensor_shape)
                dtype = mybir.dt.np(alloc.dtype)
                out_names.append(name)
                out_avals.append(jax.core.ShapedArray(shape, dtype))
        self.in_names = list(in_names)
        self.out_names = out_names
        self.out_avals = out_avals
        n_params = len(in_names)
        n_outs = len(out_avals)
        bind_names = in_names + out_names
        if partition_name is not None:
            bind_names = bind_names + [partition_name]

        def _body(*args):
            operands = list(args)
            if partition_name is not None:
                operands.append(b2j.partition_id_tensor())
            outs = b2j._bass_exec_p.bind(
                *operands,
                out_avals=tuple(out_avals),
                in_names=tuple(bind_names),
                out_names=tuple(out_names),
                lowering_input_output_aliases=(),
                sim_require_finite=True,
                sim_require_nnan=True,
                nc=nc,
            )
            return tuple(outs)

        devices = jax.devices()[:N_CORES]
        mesh = Mesh(np.asarray(devices), ("core",))
        in_specs = (PartitionSpec("core"),) * (n_params + n_outs)
        out_specs = (PartitionSpec("core"),) * n_outs
        self.sharded = jax.jit(
            shard_map(_body, mesh=mesh, in_specs=in_specs, out_specs=out_specs,
                      check_rep=False),
            keep_unused=True,
        )
        shard_sp = jax.sharding.NamedSharding(mesh, PartitionSpec("core"))
        self.dev_dummy = [
            jax.device_put(
                np.zeros((N_CORES * a.shape[0], *a.shape[1:]), a.dtype), shard_sp)
            for a in self.out_avals
        ]
        from concurrent.futures import ThreadPoolExecutor
        self._pool = ThreadPoolExecutor(max_workers=N_CORES)

    def _fetch(self, arr):
        shards = arr.addressable_shards
        parts = list(self._pool.map(
            lambda s: ((s.index[0].start or 0), np.asarray(s.data)), shards))
        parts.sort(key=lambda t: t[0])
        return np.concatenate([p[1] for p in parts], axis=0)

    def run_global(self, global_in):
        concat_in = [np.ascontiguousarray(global_in[n]) for n in self.in_names]
        outs = self.sharded(*concat_in, *self.dev_dummy)
        fetched = [self._fetch(outs[i]).reshape(N_CORES, *self.out_avals[i].shape)
                   for i in range(len(self.out_names))]
        return [
            {name: fetched[i][c] for i, name in enumerate(self.out_names)}
            for c in range(N_CORES)
        ]

    def __call__(self, maps):
        return self.run_global({
            n: np.concatenate([np.asarray(m[n]) for m in maps], axis=0)
            for n in self.in_names
        })


_RUNNERS = {}


def _get_runner(cfg):
    key = (cfg["N"], cfg["E"], cfg["NT2"], hash(cfg["tiles"]))
    r = _RUNNERS.get(key)
    if r is None:
        r = _Runner(cfg)
        _RUNNERS[key] = r
    return r


# ----------------------------------------------------------------------------
# entry point
# ----------------------------------------------------------------------------

def _in_maps(cfg, per_core):
    return [{k: v[c] for k, v in per_core.items()} for c in range(N_CORES)]


def _global_in(cfg, per_core):
    return {k: np.ascontiguousarray(v.reshape(v.shape[0] * v.shape[1],
                                              *v.shape[2:]))
            for k, v in per_core.items()}


def _assemble(cfg, asm, core_outs):
    stacked = np.stack([co["out"] for co in core_outs])      # [8, 2, SHARD]
    out = stacked[asm["node_core"], :, asm["node_pos"]]       # [N, 2]
    return np.ascontiguousarray(out.astype(np.float32))


def kernel(**inputs):
    cfg, per_core, asm = _prep(inputs)
    runner = _get_runner(cfg)
    res = runner.run_global(_global_in(cfg, per_core))
    return _assemble(cfg, asm, res)


# revision 6
# speedup vs baseline: 1.1387x; 1.1387x over previous
"""BotRGCN on 8 Trainium2 NeuronCores (Bass/Tile) — wire-optimized.

The wall-clock is dominated by the axon tunnel (~25-50 MB/s raw, no
compression, ~0.1s fixed dispatch+fetch): the kernel minimizes raw wire
bytes and ships them in two large arrays.

  - Host: encoder MLP in f32; x quantized AFFINE per-feature to 127
    levels and bit-packed 7-bit (8 codes -> 7 byte-planes of 16 cols,
    node-major).  The induced feature permutation is folded into the
    weights on host.  Device unpacks with shift/or ops on contiguous
    [128,16] slices; the gather table holds raw integer codes (exact in
    bf16), dequant (s, zp) happens after aggregation.
  - Graph: nodes dealt to 8*BPC blocks of 128 dst lanes (serpentine on
    degree).  Edges bucketed per (dst block, rel); rel is implicit in
    the tile index (per-(block,rel) tile counts, variable; blocks
    sorted per core by bucket size so the SPMD max across cores stays
    tight).  Per edge slot 3 bytes in separate planes: row_lo, row_mid,
    lane|row_hi<<7.  Padding slots point to a reserved all-zero table
    row (code 0 contributes nothing).
  - Mean normalization: per-(lane,rel) counts ship as u8; device takes
    f32 reciprocal (exact) and applies it via a rank-1 outer-product
    matmul + tensor_tensor after PSUM aggregation; affine dequant is
    one fused tensor_scalar.
  - Edge planes + counts + weights (bf16, replicated) + f32 biases/
    scales ride in ONE merged u8 "meta" array per core (bitcast slices
    on device) — few large transfers beat many small ones.
  - Numerics: aggregation exact (int codes / 0-1 selectors in bf16,
    f32 PSUM); transforms + head in f32 (weights rounded to bf16 on the
    wire only); inter-layer table in bf16.  Measured end-to-end error:
    l2 6.5e-3, max 1.6e-2 (gate 2e-2).
"""

import numpy as np
import ml_dtypes

import jax
from jax.sharding import Mesh, PartitionSpec
from jax.experimental.shard_map import shard_map

import concourse.bacc as bacc
import concourse.bass as bass
import concourse.bass2jax as b2j
import concourse.mybir as mybir
import concourse.tile as tile
from concourse.masks import make_identity

F32 = mybir.dt.float32
BF16 = mybir.dt.bfloat16
I32 = mybir.dt.int32
U8 = mybir.dt.uint8
BF = ml_dtypes.bfloat16

N_CORES = 8
D = 128
R = 2
ALPHA = 0.01
WCOLS = 524  # Wroot|Wrel0|Wrel1|Wo1 (4*128) | Wo2 (2) | f32 brg|bo1|bo2|s|zp (10 bf16)


def _lrelu_np(v):
    return np.where(v > 0, v, np.float32(ALPHA) * v)


# ----------------------------------------------------------------------------
# host-side preprocessing
# ----------------------------------------------------------------------------

def _prep(inputs):
    src = np.asarray(inputs["edge_index"][0], dtype=np.int64)
    dst = np.asarray(inputs["edge_index"][1], dtype=np.int64)
    rel = np.asarray(inputs["edge_type"], dtype=np.int64)
    N = int(np.asarray(inputs["des"]).shape[0])
    E = src.shape[0]

    BPC = ((-(-N // N_CORES)) + 1 + 127) // 128  # >=1 spare slot per core
    SHARD = BPC * 128
    NBLK = N_CORES * BPC
    TROWS = N_CORES * SHARD
    assert TROWS <= (1 << 17)

    cnt2 = np.bincount(dst * R + rel, minlength=N * R).reshape(N, R)
    deg = cnt2.sum(1)

    # serpentine deal by degree
    order = np.argsort(-deg, kind="stable")
    idx = np.arange(N)
    rnd = idx // NBLK
    pos = idx % NBLK
    blk = np.where(rnd % 2 == 0, pos, NBLK - 1 - pos)
    node_block = np.empty(N, np.int64)
    node_lane = np.empty(N, np.int64)
    node_block[order] = blk
    node_lane[order] = rnd
    assert node_lane.max() <= 127

    # per-core block relabel: sort blocks by bucket size so per-position
    # maxima across cores stay tight (variable tile counts)
    key0 = node_block[dst]
    cnt_b0 = np.bincount(key0, minlength=NBLK)
    newid = np.empty(NBLK, np.int64)
    for c in range(N_CORES):
        sl = slice(c * BPC, (c + 1) * BPC)
        rank = np.argsort(-cnt_b0[sl], kind="stable")
        newid[c * BPC + rank] = c * BPC + np.arange(BPC)
    node_block = newid[node_block]

    # relocate nodes off the reserved (new last block of core, lane 127)
    # slots — these rows must stay all-zero in both gather tables
    used = np.zeros((NBLK, 128), bool)
    used[node_block, node_lane] = True
    res_blocks = np.arange(N_CORES) * BPC + BPC - 1
    hit = np.where(np.isin(node_block, res_blocks) & (node_lane == 127))[0]
    free_mask = ~used
    free_mask[res_blocks, 127] = False
    free_list = np.argwhere(free_mask)
    assert len(free_list) >= len(hit)
    for j, i in enumerate(hit):
        node_block[i], node_lane[i] = free_list[j]

    node_core = node_block // BPC
    node_pos = (node_block % BPC) * 128 + node_lane
    node_row = node_core * SHARD + node_pos
    res_row = np.arange(N_CORES) * SHARD + SHARD - 1  # [8]

    key = node_block[dst]
    cnt_br = np.bincount(key * R + rel, minlength=NBLK * R)  # (block, rel)
    cnt_pos = cnt_br.reshape(N_CORES, BPC, R)
    Tbr = np.maximum(-(-cnt_pos.max(axis=0) // 128), 1)  # [BPC, R]
    tile_off = np.zeros(BPC * R + 1, np.int64)
    tile_off[1:] = np.cumsum(Tbr.reshape(-1))
    NT2 = int(tile_off[-1])

    lane_e = node_lane[dst]
    rowg = node_row[src]
    order_e = np.lexsort((rowg, lane_e, rel, key))
    ks, rs = key[order_e], rel[order_e]
    seg = ks * R + rs
    starts = np.zeros(NBLK * R, np.int64)
    starts[1:] = np.cumsum(cnt_br)[:-1]
    pos_in = np.arange(E) - starts[seg]

    core_e = ks // BPC
    posb_e = ks % BPC
    col_e = tile_off[posb_e * R + rs] + pos_in // 128
    par_e = pos_in % 128

    rows_all = np.broadcast_to(res_row[:, None, None], (N_CORES, NT2, 128)).copy()
    lanes_all = np.zeros((N_CORES, NT2, 128), np.int64)
    rows_all[core_e, col_e, par_e] = rowg[order_e]
    lanes_all[core_e, col_e, par_e] = lane_e[order_e]

    def planes(a):
        return np.ascontiguousarray(a.transpose(0, 2, 1)).astype(np.uint8)

    ep0 = planes(rows_all & 255)
    ep1 = planes((rows_all >> 8) & 255)
    ep2 = planes(lanes_all | ((rows_all >> 16) << 7))

    cnt_nl = np.zeros((NBLK, 128, R), np.int64)
    cnt_nl[node_block, node_lane] = cnt2
    assert cnt_nl.max() <= 255
    # [core, lane, r*BPC + b] u8
    cntw = np.ascontiguousarray(
        np.maximum(cnt_nl, 1).reshape(N_CORES, BPC, 128, R)
        .transpose(0, 2, 3, 1).reshape(N_CORES, 128, R * BPC)).astype(np.uint8)

    # host encoder + affine 127-level quantization
    g = lambda k: np.asarray(inputs[k], np.float32)
    d = _lrelu_np(g("des") @ g("W_des") + g("b_des"))
    t = _lrelu_np(g("tweet") @ g("W_tweet") + g("b_tweet"))
    n = _lrelu_np(g("num_prop") @ g("W_num") + g("b_num"))
    c = _lrelu_np(g("cat_prop") @ g("W_cat") + g("b_cat"))
    x = _lrelu_np(np.concatenate([d, t, n, c], axis=1) @ g("W_in") + g("b_in"))
    lo, hi = x.min(0), x.max(0)
    xs = np.maximum(hi - lo, 1e-12).astype(np.float32) / 127.0
    zp = lo.astype(np.float32)
    codes = np.clip(np.rint((x - zp) / xs), 0, 127).astype(np.uint8)

    # feature permutation induced by the 7-bit plane decode:
    # new feature 16*k + g  <-  original feature 8*g + k
    kk, gg = np.meshgrid(np.arange(8), np.arange(16), indexing="ij")
    perm = (8 * gg + kk).reshape(-1)  # [128] new -> old

    # 7-bit pack: group g = original features 8g..8g+7 -> 7 bytes, laid
    # as 7 planes of 16 cols (wire col = 16*p + g)
    acc = np.zeros((N, 16), np.uint64)
    for k in range(8):
        acc |= codes[:, k::8].astype(np.uint64) << np.uint64(7 * k)
    x7 = np.empty((N, 112), np.uint8)
    for p in range(7):
        x7[:, 16 * p:16 * (p + 1)] = (acc >> np.uint64(8 * p)).astype(np.uint8)

    row_node = np.full(TROWS, -1, np.int64)
    row_node[node_row] = np.arange(N)
    valid = row_node >= 0
    Xr = x7[np.where(valid, row_node, 0)]
    Xr[~valid] = 0
    # [core, node%128, (node//128)*112 + byte]: ingest chunk k is the
    # column slice [:, 112k:112(k+1)] — lets x ride in the single blob
    xqp = np.ascontiguousarray(
        Xr.reshape(N_CORES, BPC, 128, 112).transpose(0, 2, 1, 3)
        .reshape(N_CORES, 128, BPC * 112))

    # weights: bf16, replicated; scales/biases f32 bitcast into bf16 cols.
    # All feature-indexed tensors permuted to the decode order.
    Wroot = g("W_root")[perm][:, perm]
    Wrel0 = g("W_rel")[0][perm][:, perm]
    Wrel1 = g("W_rel")[1][perm][:, perm]
    Wo1 = g("W_o1")[perm]
    Wo2 = g("W_o2")
    wcore = np.zeros((128, WCOLS), BF)
    wcore[:, 0:128] = Wroot.astype(BF)
    wcore[:, 128:256] = Wrel0.astype(BF)
    wcore[:, 256:384] = Wrel1.astype(BF)
    wcore[:, 384:512] = Wo1.astype(BF)
    wcore[:, 512:514] = Wo2.astype(BF)
    f32sec = np.zeros((128, 5), np.float32)
    f32sec[:, 0] = g("b_rgcn")[perm]
    f32sec[:, 1] = g("b_o1")
    f32sec[0:2, 2] = g("b_o2")
    f32sec[:, 3] = xs[perm]
    f32sec[:, 4] = zp[perm]
    wcore[:, 514:524] = f32sec.view(BF)
    wts = np.broadcast_to(wcore, (N_CORES, 128, WCOLS)).copy()

    # merge EVERYTHING (x codes + edge planes + counts + weights) into a
    # single wire array per core: each transfer call costs ~80ms latency
    WOFF = ((3 * NT2 + 2 * BPC + 3) // 4) * 4
    M = WOFF + 2 * WCOLS
    meta = np.zeros((N_CORES, 128, M), np.uint8)
    meta[:, :, 0:NT2] = ep0
    meta[:, :, NT2:2 * NT2] = ep1
    meta[:, :, 2 * NT2:3 * NT2] = ep2
    meta[:, :, 3 * NT2:3 * NT2 + 2 * BPC] = cntw
    meta[:, :, WOFF:] = wts.view(np.uint8)
    blob = np.ascontiguousarray(np.concatenate([xqp, meta], axis=2))

    cfg = dict(N=N, E=E, BPC=BPC, SHARD=SHARD, TROWS=TROWS, NT2=NT2,
               tiles=tuple(map(tuple, Tbr.tolist())))
    per_core = dict(blob=blob)
    asm = dict(node_core=node_core, node_pos=node_pos)
    return cfg, per_core, asm


# ----------------------------------------------------------------------------
# device program
# ----------------------------------------------------------------------------

def _enc_slices(shard, w_max=512):
    out, c = [], 0
    while c < shard:
        w = min(w_max, shard - c)
        out.append((c, w))
        c += w
    return out


def build_bass(cfg, sim_compat=False):
    BPC, SHARD, TROWS, NT2 = cfg["BPC"], cfg["SHARD"], cfg["TROWS"], cfg["NT2"]
    tiles = cfg["tiles"]
    tile_off = [0]
    for b in range(BPC):
        for r in range(R):
            tile_off.append(tile_off[-1] + tiles[b][r])

    nc = bacc.Bacc("TRN2", target_bir_lowering=False, debug=False,
                   num_devices=N_CORES)

    XOFF = BPC * 112
    WOFF = XOFF + ((3 * NT2 + 2 * BPC + 3) // 4) * 4
    M = WOFF + 2 * WCOLS
    blob = nc.dram_tensor("blob", [128, M], U8, kind="ExternalInput")
    out = nc.dram_tensor("out", [2, SHARD], BF16, kind="ExternalOutput")
    ep0 = blob[:, XOFF:XOFF + NT2]
    ep1 = blob[:, XOFF + NT2:XOFF + 2 * NT2]
    ep2 = blob[:, XOFF + 2 * NT2:XOFF + 3 * NT2]
    cnt = blob[:, XOFF + 3 * NT2:XOFF + 3 * NT2 + 2 * BPC]
    wtsb = blob[:, WOFF:WOFF + 2 * 514].bitcast(BF16)
    wtsf = blob[:, WOFF + 2 * 514:WOFF + 2 * 514 + 20].bitcast(F32)

    groups = [list(range(N_CORES))]
    AG = "AllGather"
    BY = mybir.AluOpType.bypass

    def _lrelu(pool, ps_ap, bias_ap, w, name):
        t = pool.tile([ps_ap.shape[0], w], F32, name=name)
        if not sim_compat:
            nc.scalar.activation(out=t[:], in_=ps_ap,
                                 func=mybir.ActivationFunctionType.Prelu,
                                 bias=bias_ap, scale=1.0, alpha=ALPHA)
            return t
        zt = pool.tile([ps_ap.shape[0], w], F32, name=name + "_z")
        nc.scalar.activation(out=zt[:], in_=ps_ap,
                             func=mybir.ActivationFunctionType.Identity,
                             bias=bias_ap, scale=1.0)
        rt = pool.tile([ps_ap.shape[0], w], F32, name=name + "_r")
        nc.scalar.activation(out=rt[:], in_=ps_ap,
                             func=mybir.ActivationFunctionType.Relu,
                             bias=bias_ap, scale=1.0)
        t1 = pool.tile([ps_ap.shape[0], w], F32, name=name + "_t1")
        nc.vector.tensor_scalar(out=t1[:], in0=zt[:], scalar1=ALPHA, scalar2=None,
                                op0=mybir.AluOpType.mult)
        t2 = pool.tile([ps_ap.shape[0], w], F32, name=name + "_t2")
        nc.vector.tensor_scalar(out=t2[:], in0=rt[:], scalar1=1.0 - ALPHA,
                                scalar2=None, op0=mybir.AluOpType.mult)
        nc.vector.tensor_tensor(out=t[:], in0=t1[:], in1=t2[:],
                                op=mybir.AluOpType.add)
        return t

    with tile.TileContext(nc) as tc:
        with tc.tile_pool(name="const", bufs=1) as cp, \
             tc.tile_pool(name="dram", bufs=1, space="DRAM") as dp:
            # ---- weights: replicated bf16 wire, f32 on device ----
            c_Wb = cp.tile([128, 514], BF16)
            nc.sync.dma_start(c_Wb[:], wtsb)
            c_W = cp.tile([128, 514], F32)
            nc.vector.tensor_copy(out=c_W[:], in_=c_Wb[:])
            c_f32 = cp.tile([128, 5], F32)
            nc.sync.dma_start(c_f32[:], wtsf)
            c_Wroot = c_W[:, 0:128]
            c_Wrel = [c_W[:, 128:256], c_W[:, 256:384]]
            c_Wo1 = c_W[:, 384:512]
            c_Wo2 = c_W[:, 512:514]
            c_brg = c_f32[:, 0:1]
            c_bo1 = c_f32[:, 1:2]
            c_bo2 = c_f32[0:2, 2:3]
            c_s = c_f32[:, 3:4]
            c_zp = c_f32[:, 4:5]

            ident = cp.tile([128, 128], BF16)
            make_identity(nc, ident[:])

            # ---- edge metadata decode ----
            with tc.tile_pool(name="dec", bufs=1) as dcp, \
                 tc.tile_pool(name="decps", bufs=2, space="PSUM") as dps:
                p0 = dcp.tile([128, NT2], U8)
                p1 = dcp.tile([128, NT2], U8)
                p2 = dcp.tile([128, NT2], U8)
                nc.sync.dma_start(p0[:], ep0)
                nc.sync.dma_start(p1[:], ep1)
                nc.sync.dma_start(p2[:], ep2)
                g0 = dcp.tile([128, NT2], I32)
                nc.vector.tensor_copy(out=g0[:], in_=p0[:])
                g1 = dcp.tile([128, NT2], I32)
                nc.vector.tensor_copy(out=g1[:], in_=p1[:])
                g2 = dcp.tile([128, NT2], I32)
                nc.vector.tensor_copy(out=g2[:], in_=p2[:])
                t1_ = dcp.tile([128, NT2], I32)
                nc.vector.tensor_scalar(out=t1_[:], in0=g1[:], scalar1=8,
                                        scalar2=None,
                                        op0=mybir.AluOpType.logical_shift_left)
                t2_ = dcp.tile([128, NT2], I32)
                nc.vector.tensor_scalar(out=t2_[:], in0=g2[:], scalar1=7,
                                        scalar2=16,
                                        op0=mybir.AluOpType.logical_shift_right,
                                        op1=mybir.AluOpType.logical_shift_left)
                c_gidx = cp.tile([128, NT2], I32)
                nc.vector.tensor_tensor(out=c_gidx[:], in0=g0[:], in1=t1_[:],
                                        op=mybir.AluOpType.add)
                nc.vector.tensor_tensor(out=c_gidx[:], in0=c_gidx[:], in1=t2_[:],
                                        op=mybir.AluOpType.add)
                lf = dcp.tile([128, NT2], I32)
                nc.vector.tensor_scalar(out=lf[:], in0=g2[:], scalar1=127,
                                        scalar2=None,
                                        op0=mybir.AluOpType.bitwise_and)
                c_lane = cp.tile([128, NT2], F32)
                nc.vector.tensor_copy(out=c_lane[:], in_=lf[:])

                cb = dcp.tile([128, R * BPC], U8)
                nc.sync.dma_start(cb[:], cnt)
                cf = dcp.tile([128, R * BPC], BF16)
                nc.vector.tensor_copy(out=cf[:], in_=cb[:])
                c_wvd = dp.tile([R * BPC, 128], F32, name="wvd")
                for r in range(R):
                    pst = dps.tile([BPC, 128], BF16, name="cwps")
                    nc.tensor.matmul(out=pst[:], lhsT=cf[:, r * BPC:(r + 1) * BPC],
                                     rhs=ident[:], is_transpose=True,
                                     start=True, stop=True)
                    cg = dcp.tile([BPC, 128], F32, name=f"cntT{r}")
                    nc.vector.tensor_copy(out=cg[:], in_=pst[:])
                    wvr = dcp.tile([BPC, 128], F32, name=f"wvT{r}")
                    nc.vector.reciprocal(out=wvr[:], in_=cg[:])
                    nc.sync.dma_start(c_wvd[r * BPC:(r + 1) * BPC, :], wvr[:])

            c_ioti = cp.tile([128, 128], I32)
            nc.gpsimd.iota(c_ioti[:], pattern=[[1, 128]], base=0,
                           channel_multiplier=0)
            c_iota = cp.tile([128, 128], F32)
            nc.vector.tensor_copy(out=c_iota[:], in_=c_ioti[:])
            c_ones = cp.tile([1, 128], F32)
            nc.vector.memset(c_ones[:], 1.0)
            zrow = cp.tile([1, 128], BF16)
            nc.vector.memset(zrow[:], 0.0)

            # DRAM intermediates
            xfm = [dp.tile([128, SHARD], F32, name=f"xfm{i}") for i in range(3)]
            xnm = [dp.tile([SHARD, 128], BF16, name=f"xnm{i}") for i in range(2)]
            tables = [dp.tile([TROWS, 128], BF16, addr_space="Shared",
                              name=f"table{i}") for i in range(2)]

            # ---- ingest: unpack 7-bit codes -> table codes + dequant x ----
            OR_ = mybir.AluOpType.bitwise_or
            AND = mybir.AluOpType.bitwise_and
            LSR = mybir.AluOpType.logical_shift_right
            LSL = mybir.AluOpType.logical_shift_left
            with tc.tile_pool(name="ing", bufs=4) as ip, \
                 tc.tile_pool(name="ingps", bufs=2, space="PSUM") as ips:
                for k in range(BPC):
                    raw = ip.tile([128, 112], U8, name="raw")
                    nc.sync.dma_start(raw[:], blob[:, k * 112:(k + 1) * 112])
                    dec = ip.tile([128, 128], U8, name="dec")
                    for j in range(8):
                        a, sa = (7 * j) >> 3, (7 * j) & 7
                        dst = dec[:, 16 * j:16 * (j + 1)]
                        if sa == 0:
                            nc.vector.tensor_scalar(
                                out=dst, in0=raw[:, 16 * a:16 * (a + 1)],
                                scalar1=127, scalar2=None, op0=AND)
                        elif a == 6:
                            nc.vector.tensor_scalar(
                                out=dst, in0=raw[:, 16 * a:16 * (a + 1)],
                                scalar1=sa, scalar2=None, op0=LSR)
                        else:
                            tlo = ip.tile([128, 16], U8, name="tlo")
                            nc.vector.tensor_scalar(
                                out=tlo[:], in0=raw[:, 16 * a:16 * (a + 1)],
                                scalar1=sa, scalar2=None, op0=LSR)
                            thi = ip.tile([128, 16], U8, name="thi")
                            nc.vector.tensor_scalar(
                                out=thi[:], in0=raw[:, 16 * (a + 1):16 * (a + 2)],
                                scalar1=8 - sa, scalar2=127, op0=LSL, op1=AND)
                            nc.vector.tensor_tensor(out=dst, in0=tlo[:],
                                                    in1=thi[:], op=OR_)
                    nb = ip.tile([128, 128], BF16, name="nb")
                    nc.vector.tensor_copy(out=nb[:], in_=dec[:])
                    nc.sync.dma_start(xnm[0][k * 128:(k + 1) * 128, :], nb[:])
                    ps_t = ips.tile([128, 128], BF16, name="ps_t")
                    nc.tensor.matmul(out=ps_t[:], lhsT=nb[:], rhs=ident[:],
                                     is_transpose=True, start=True, stop=True)
                    fm = ip.tile([128, 128], F32, name="fm")
                    nc.vector.tensor_scalar(out=fm[:], in0=ps_t[:],
                                            scalar1=c_s, scalar2=c_zp,
                                            op0=mybir.AluOpType.mult,
                                            op1=mybir.AluOpType.add)
                    nc.sync.dma_start(xfm[0][:, k * 128:(k + 1) * 128], fm[:])

            nc.gpsimd.collective_compute(AG, BY, replica_groups=groups,
                                         ins=[xnm[0].opt()], outs=[tables[0].opt()])

            # ---- rgcn layers ----
            for L in range(2):
                table = tables[L]
                Wroot_L, Wrel_L, bias_L = c_Wroot, c_Wrel, c_brg
                with tc.tile_pool(name=f"gp{L}", bufs=16) as gp, \
                     tc.tile_pool(name=f"sp{L}", bufs=8) as sp, \
                     tc.tile_pool(name=f"up{L}", bufs=4) as up, \
                     tc.tile_pool(name=f"Sps{L}", bufs=2, space="PSUM") as Sps, \
                     tc.tile_pool(name=f"Wps{L}", bufs=2, space="PSUM") as Wps, \
                     tc.tile_pool(name=f"aps{L}", bufs=2, space="PSUM") as aps, \
                     tc.tile_pool(name=f"tps{L}", bufs=2, space="PSUM") as tps:
                    n_units = BPC // 2 + (BPC % 2)
                    for u in range(n_units):
                        blocks = [b for b in (2 * u, 2 * u + 1) if b < BPC]
                        Us = []
                        for b in blocks:
                            ps = Sps.tile([128, 256], F32, name="psS")
                            for r in range(R):
                                Tb = tiles[b][r]
                                base = tile_off[b * R + r]
                                for t in range(Tb):
                                    T = base + t
                                    G = gp.tile([128, 128], BF16, name="G")
                                    nc.gpsimd.indirect_dma_start(
                                        out=G[:], out_offset=None, in_=table[:],
                                        in_offset=bass.IndirectOffsetOnAxis(
                                            ap=c_gidx[:, T:T + 1], axis=0))
                                    sel = sp.tile([128, 128], BF16, name="sel")
                                    nc.vector.tensor_scalar(
                                        out=sel[:], in0=c_iota[:],
                                        scalar1=c_lane[:, T:T + 1], scalar2=None,
                                        op0=mybir.AluOpType.is_equal)
                                    nc.tensor.matmul(
                                        out=ps[:, r * 128:(r + 1) * 128],
                                        lhsT=G[:], rhs=sel[:],
                                        start=(t == 0), stop=(t == Tb - 1))
                            U = up.tile([128, 256], F32, name="U")
                            for r in range(R):
                                wrow = sp.tile([1, 128], F32, name="wrow")
                                nc.sync.dma_start(wrow[:],
                                                  c_wvd[r * BPC + b:r * BPC + b + 1, :])
                                wvt = Wps.tile([128, 128], F32, name="wvt")
                                nc.tensor.matmul(
                                    out=wvt[:], lhsT=c_ones[:], rhs=wrow[:],
                                    start=True, stop=True)
                                wvs = sp.tile([128, 128], F32, name="wvs")
                                nc.vector.tensor_copy(out=wvs[:], in_=wvt[:])
                                nc.vector.tensor_tensor(
                                    out=U[:, r * 128:(r + 1) * 128],
                                    in0=ps[:, r * 128:(r + 1) * 128],
                                    in1=wvs[:], op=mybir.AluOpType.mult)
                            if L == 0:
                                # dequant the aggregated code-means
                                Ud = up.tile([128, 256], F32, name="Ud")
                                nc.vector.tensor_scalar(
                                    out=Ud[:], in0=U[:], scalar1=c_s,
                                    scalar2=c_zp, op0=mybir.AluOpType.mult,
                                    op1=mybir.AluOpType.add)
                                U = Ud
                            Us.append(U)
                        w = 128 * len(blocks)
                        c0 = u * 256
                        xr = up.tile([128, w], F32, name="xr")
                        nc.sync.dma_start(xr[:], xfm[L][:, c0:c0 + w])
                        agg = aps.tile([128, w], F32, name="agg")
                        nc.tensor.matmul(out=agg[:], lhsT=Wroot_L, rhs=xr[:],
                                         start=True, stop=False)
                        for h, b in enumerate(blocks):
                            last = (h == len(blocks) - 1)
                            for r in range(R):
                                nc.tensor.matmul(
                                    out=agg[:, h * 128:(h + 1) * 128],
                                    lhsT=Wrel_L[r],
                                    rhs=Us[h][:, r * 128:(r + 1) * 128],
                                    start=False,
                                    stop=(last and r == R - 1))
                        y = up.tile([128, w], F32, name="y")
                        nc.scalar.activation(out=y[:], in_=agg[:],
                                             func=mybir.ActivationFunctionType.Identity,
                                             bias=bias_L, scale=1.0)
                        nc.sync.dma_start(xfm[L + 1][:, c0:c0 + w], y[:])
                        if L == 0:
                            yb = up.tile([128, w], BF16, name="yb")
                            nc.vector.tensor_copy(out=yb[:], in_=y[:])
                            for j in range(len(blocks)):
                                ps_t = tps.tile([128, 128], BF16, name="ps_t2")
                                nc.tensor.matmul(
                                    out=ps_t[:],
                                    lhsT=yb[:, j * 128:(j + 1) * 128],
                                    rhs=ident[:], is_transpose=True,
                                    start=True, stop=True)
                                tr_t = up.tile([128, 128], BF16, name="tr2")
                                nc.vector.tensor_copy(out=tr_t[:], in_=ps_t[:])
                                nc.sync.dma_start(
                                    xnm[1][c0 + j * 128:c0 + (j + 1) * 128, :],
                                    tr_t[:])
                if L == 0:
                    nc.sync.dma_start(xnm[1][SHARD - 1:SHARD, :], zrow[:])
                    nc.gpsimd.collective_compute(AG, BY, replica_groups=groups,
                                                 ins=[xnm[1].opt()],
                                                 outs=[tables[1].opt()])

            # ---- head ----
            with tc.tile_pool(name="hd", bufs=3) as hp, \
                 tc.tile_pool(name="hps", bufs=2, space="PSUM") as hps, \
                 tc.tile_pool(name="ops", bufs=2, space="PSUM") as ops:
                for (c0, w) in _enc_slices(SHARD):
                    xt = hp.tile([128, w], F32, name="xt")
                    nc.sync.dma_start(xt[:], xfm[2][:, c0:c0 + w])
                    ps_h = hps.tile([128, w], F32, name="ps_h")
                    nc.tensor.matmul(out=ps_h[:], lhsT=c_Wo1, rhs=xt[:],
                                     start=True, stop=True)
                    z_t = _lrelu(hp, ps_h[:], c_bo1, w, "z_t")
                    ps_o = ops.tile([2, w], F32, name="ps_o")
                    nc.tensor.matmul(out=ps_o[:], lhsT=c_Wo2, rhs=z_t[:],
                                     start=True, stop=True)
                    o_t = hp.tile([2, w], BF16, name="o_t")
                    nc.scalar.activation(out=o_t[:], in_=ps_o[:],
                                         func=mybir.ActivationFunctionType.Identity,
                                         bias=c_bo2, scale=1.0)
                    nc.sync.dma_start(out[:, c0:c0 + w], o_t[:])
    nc.compile()
    return nc


# ----------------------------------------------------------------------------
# cached PJRT runner (unchanged from v1)
# ----------------------------------------------------------------------------

class _Runner:
    def __init__(self, cfg):
        self.cfg = cfg
        self.nc = build_bass(cfg)
        b2j.install_neuronx_cc_hook()
        nc = self.nc
        partition_name = (nc.partition_id_tensor.name
                          if nc.partition_id_tensor else None)
        in_names, out_names, out_avals = [], [], []
        for alloc in nc.m.functions[0].allocations:
            if not isinstance(alloc, mybir.MemoryLocationSet):
                continue
            name = alloc.memorylocations[0].name
            if alloc.kind == "ExternalInput":
                if name != partition_name:
                    in_names.append(name)
            elif alloc.kind == "ExternalOutput":
                shape = tuple(alloc.tensor_shape)
                dtype = mybir.dt.np(alloc.dtype)
                out_names.append(name)
                out_avals.append(jax.core.ShapedArray(shape, dtype))
        self.in_names = list(in_names)
        self.out_names = out_names
        self.out_avals = out_avals
        n_params = len(in_names)
        n_outs = len(out_avals)
        bind_names = in_names + out_names
        if partition_name is not None:
            bind_names = bind_names + [partition_name]

        def _body(*args):
            operands = list(args)
            if partition_name is not None:
                operands.append(b2j.partition_id_tensor())
            outs = b2j._bass_exec_p.bind(
                *operands,
                out_avals=tuple(out_avals),
                in_names=tuple(bind_names),
                out_names=tuple(out_names),
                lowering_input_output_aliases=(),
                sim_require_finite=True,
                sim_require_nnan=True,
                nc=nc,
            )
            return tuple(outs)

        devices = jax.devices()[:N_CORES]
        mesh = Mesh(np.asarray(devices), ("core",))
        in_specs = (PartitionSpec("core"),) * (n_params + n_outs)
        out_specs = (PartitionSpec("core"),) * n_outs
        self.sharded = jax.jit(
            shard_map(_body, mesh=mesh, in_specs=in_specs, out_specs=out_specs,
                      check_rep=False),
            keep_unused=True,
        )
        shard_sp = jax.sharding.NamedSharding(mesh, PartitionSpec("core"))
        self.dev_dummy = [
            jax.device_put(
                np.zeros((N_CORES * a.shape[0], *a.shape[1:]), a.dtype), shard_sp)
            for a in self.out_avals
        ]
        from concurrent.futures import ThreadPoolExecutor
        self._pool = ThreadPoolExecutor(max_workers=N_CORES)

    def _fetch(self, arr):
        shards = arr.addressable_shards
        parts = list(self._pool.map(
            lambda s: ((s.index[0].start or 0), np.asarray(s.data)), shards))
        parts.sort(key=lambda t: t[0])
        return np.concatenate([p[1] for p in parts], axis=0)

    def run_global(self, global_in):
        concat_in = [np.ascontiguousarray(global_in[n]) for n in self.in_names]
        outs = self.sharded(*concat_in, *self.dev_dummy)
        fetched = [self._fetch(outs[i]).reshape(N_CORES, *self.out_avals[i].shape)
                   for i in range(len(self.out_names))]
        return [
            {name: fetched[i][c] for i, name in enumerate(self.out_names)}
            for c in range(N_CORES)
        ]

    def __call__(self, maps):
        return self.run_global({
            n: np.concatenate([np.asarray(m[n]) for m in maps], axis=0)
            for n in self.in_names
        })


_RUNNERS = {}


def _get_runner(cfg):
    key = (cfg["N"], cfg["E"], cfg["NT2"], hash(cfg["tiles"]))
    r = _RUNNERS.get(key)
    if r is None:
        r = _Runner(cfg)
        _RUNNERS[key] = r
    return r


# ----------------------------------------------------------------------------
# entry point
# ----------------------------------------------------------------------------

def _in_maps(cfg, per_core):
    return [{k: v[c] for k, v in per_core.items()} for c in range(N_CORES)]


def _global_in(cfg, per_core):
    return {k: np.ascontiguousarray(v.reshape(v.shape[0] * v.shape[1],
                                              *v.shape[2:]))
            for k, v in per_core.items()}


def _assemble(cfg, asm, core_outs):
    stacked = np.stack([co["out"] for co in core_outs])      # [8, 2, SHARD]
    out = stacked[asm["node_core"], :, asm["node_pos"]]       # [N, 2]
    return np.ascontiguousarray(out.astype(np.float32))


def kernel(**inputs):
    cfg, per_core, asm = _prep(inputs)
    runner = _get_runner(cfg)
    res = runner.run_global(_global_in(cfg, per_core))
    return _assemble(cfg, asm, res)
